# revision 17
# baseline (speedup 1.0000x reference)
"""CPR linear (int8-dequant matmul with column reordering) on 8 Trainium2
NeuronCores.

Math: y = x[:, col_indices] @ (W_int8 * repeat(scales, gs)) + bias
Equivalently, with inv = argsort(col_indices):
    y[m, n] = sum_j x[m, j] * W[inv[j], n] * scales[inv[j]//gs, n]
so x is consumed in natural column order and the permutation rides on W's
rows (host-side index gather; W is 8x smaller than x). The dequantized
weight wd = W_perm * scales is formed on the host in f32 and shipped as
bf16 — exactly what the device-side dequant produced, minus 4MB of scale
DMA and a DVE pass.

Sharding: column-parallel. Each core owns 512 output features: its slice
of wd and bias; x is replicated.

Per-core device kernel:
  - bias broadcast [512] -> [128, 512] via DMA
  - wd resident 4MB in SBUF, loaded in k-chunks (smallest first, split
    across two DMA queues) so the first matmuls gate on a 0.125MB load
  - main loop over 8 m-blocks of 1024 rows:
      8 DMA loads  x[kg, mb] -> xT [128k, 4, 1024m] bf16
      8 m-subtiles x 32 k-tiles accumulating matmuls into PSUM [128, 512]
      PSUM + bias -> SBUF (bf16) -> DMA out (host upcasts to f32)
"""
from contextlib import ExitStack

import numpy as np
import ml_dtypes

import concourse.bass as bass
import concourse.bacc as bacc
import concourse.mybir as mybir
import concourse.tile as tile

B, S, K, N = 4, 2048, 4096, 4096
M = B * S                    # 8192
NCORES = 8
NS = N // NCORES             # 512 output cols per core
P = 128
NKT = K // P                 # 32 k-tiles
MB = 512                     # m-block rows
NMB = M // MB                # 16
MSUB = MB // P               # 4

bf16 = mybir.dt.bfloat16
f32 = mybir.dt.float32


KB = 4                       # k-tiles batched per x-load DMA (1MB transfers)
NKG = NKT // KB              # 8 k-groups


def build(repeats: int = 1, variant: str = "full"):
    """variant: "full" | "nomm" (DMA/DVE path only) | "mmonly" (PE path only)
    | "mmonly256" (PE path, half-width moving operand) | "halfm" (first half
    of the m-blocks only) | "ktint" (full, kt-interleaved psum chains)"""
    do_mm = variant in ("full", "mmonly", "mmonly256", "halfm", "quartm", "ktint")
    do_xdma = variant in ("full", "nomm", "halfm", "quartm", "ktint")
    nw = 256 if variant == "mmonly256" else NS
    nmb = {"halfm": NMB // 2, "quartm": NMB // 4}.get(variant, NMB)

    nc = bacc.Bacc(None)
    # x supplied pre-transposed [K, M] bf16 (host does cast + transpose)
    x_d = nc.dram_tensor("xbf", [K, M], bf16, kind="ExternalInput")
    w_d = nc.dram_tensor("wdbf", [K, NS], bf16, kind="ExternalInput")
    y_d = nc.dram_tensor("y", [M, NS], bf16, kind="ExternalOutput")

    with tile.TileContext(nc) as tc, ExitStack() as stk:
        if repeats > 1:
            stk.enter_context(tc.For_i(0, repeats, 1))
        with (
            tc.tile_pool(name="consts", bufs=1) as consts,
            tc.tile_pool(name="wpool", bufs=2) as wpool,
            tc.tile_pool(name="xpool", bufs=2) as xpool,
            tc.tile_pool(name="opool", bufs=2) as opool,
            tc.tile_pool(name="psum", bufs=8, space="PSUM") as psum_pool,
        ):
            # dequantized weights, resident: [128, NKT*NS] bf16 (4MB).
            # Loaded in k-chunks, smallest first, alternating between two
            # DMA queues, so the first matmuls gate on only a 0.125MB load
            # and the full tensor lands in ~half the single-queue time.
            wd = wpool.tile([P, NKT * NS], bf16, tag="wd")
            W_CHUNKS = [1, 1, 2, 2, 4, 4, 6, 6, 6]
            queues = [nc.scalar, nc.gpsimd]
            k0 = 0
            for h, H in enumerate(W_CHUNKS):
                r = slice(k0 * P, (k0 + H) * P)
                queues[h % 2].dma_start(
                    out=wd[:, k0 * NS:(k0 + H) * NS].rearrange(
                        "p (t n) -> p t n", n=NS),
                    in_=w_d[r, :].rearrange("(t p) n -> p t n", p=P))
                k0 += H

            xT_static = None
            if not do_xdma:
                xT_static = []
                for kg in range(NKG):
                    ts_tile = consts.tile([P, KB, MB], bf16, tag=f"xTs{kg}")
                    nc.vector.memset(ts_tile, 0.5)
                    xT_static.append(ts_tile)

            for mb in range(nmb):
                m0 = mb * MB
                if do_xdma:
                    xT = []
                    for kg in range(NKG):
                        t = xpool.tile([P, KB, MB], bf16, tag=f"xT{kg}")
                        src = x_d[kg * KB * P:(kg + 1) * KB * P, m0:m0 + MB]
                        q = nc.scalar if (mb == 0 and kg >= 6) else nc.sync
                        q.dma_start(
                            out=t, in_=src.rearrange("(b p) m -> p b m", p=P),
                        )
                        xT.append(t)
                else:
                    xT = xT_static
                if not do_mm:
                    continue
                # process m-subtiles in pairs: two PSUM banks accumulate,
                # both evict into one [128, 2, nw] tile, one 256KB store
                # all 4 m-subtiles of the block accumulate concurrently
                # in 4 PSUM banks, kt-interleaved; one 512KB store per block
                ot = opool.tile([P, MSUB, nw], bf16, tag="ot")
                ps_0 = psum_pool.tile([P, nw], f32, tag="ps")
                ps_1 = psum_pool.tile([P, nw], f32, tag="ps")
                ps_2 = psum_pool.tile([P, nw], f32, tag="ps")
                ps_3 = psum_pool.tile([P, nw], f32, tag="ps")
                pss = [ps_0, ps_1, ps_2, ps_3]
                for kt in range(NKT):
                    for ms in range(MSUB):
                        nc.tensor.matmul(
                            pss[ms],
                            xT[kt // KB][:, kt % KB, ms * P:(ms + 1) * P],
                            wd[:, kt * NS:kt * NS + nw],
                            start=(kt == 0), stop=(kt == NKT - 1),
                        )
                # plain copies (bias is added host-side), split across
                # DVE and ACT so the 4 bank evictions drain in half the time
                for ms in range(MSUB):
                    if ms % 2 == 0:
                        nc.vector.tensor_copy(out=ot[:, ms], in_=pss[ms])
                    else:
                        nc.scalar.copy(out=ot[:, ms], in_=pss[ms])
                dst = y_d[m0:m0 + MB, :nw]
                nc.scalar.dma_start(
                    out=dst.rearrange("(b p) n -> p b n", p=P), in_=ot,
                )

    nc.compile()
    return nc


def make_in_maps(x, scales, bias, weight_int8, col_indices, group_size):
    """Host-side sharding/layout prep: index gathers, dequant, dtype casts."""
    gs = int(group_size)
    x2 = np.asarray(x, dtype=np.float32).reshape(M, K)
    x_bf = x2.T.astype(ml_dtypes.bfloat16, order="C")   # [K, M], bf16

    ci = np.asarray(col_indices).astype(np.int64)
    inv = np.argsort(ci)                     # inv[j]: W row paired with x col j
    gi = inv // gs                           # scale group per permuted row

    sc = np.asarray(scales, dtype=np.float32)
    # wd[j, n] = W[inv[j], n] * scales[inv[j]//gs, n], formed in f32
    wd = np.asarray(weight_int8)[inv].astype(np.float32) * sc[gi]
    wd = wd.astype(ml_dtypes.bfloat16)
    bias = np.asarray(bias, dtype=np.float32)

    in_maps = []
    for c in range(NCORES):
        cols = slice(c * NS, (c + 1) * NS)
        in_maps.append({
            "xbf": x_bf,
            "wdbf": np.ascontiguousarray(wd[:, cols]),
        })
    return in_maps


_RUNNER = None


def _make_runner():
    """Build the bass module once and wrap it in a cached sharded jit."""
    import jax
    from jax.sharding import Mesh, PartitionSpec, NamedSharding
    from jax.experimental.shard_map import shard_map
    from concourse import bass2jax
    from concourse.bass2jax import _bass_exec_p, install_neuronx_cc_hook

    nc = build(repeats=1)
    install_neuronx_cc_hook()
    partition_name = nc.partition_id_tensor.name if nc.partition_id_tensor else None

    in_names, out_names, out_avals, zero_outs = [], [], [], []
    for alloc in nc.m.functions[0].allocations:
        if not isinstance(alloc, mybir.MemoryLocationSet):
            continue
        name = alloc.memorylocations[0].name
        if alloc.kind == "ExternalInput":
            if name != partition_name:
                in_names.append(name)
        elif alloc.kind == "ExternalOutput":
            out_names.append(name)
            shape = tuple(alloc.tensor_shape)
            dtype = mybir.dt.np(alloc.dtype)
            out_avals.append(jax.core.ShapedArray(shape, dtype))
            zero_outs.append(np.zeros(shape, dtype))
    all_in_names = list(in_names) + list(out_names)
    if partition_name is not None:
        all_in_names.append(partition_name)
    n_params, n_outs = len(in_names), len(out_names)

    def _body(*args):
        operands = list(args)
        if partition_name is not None:
            operands.append(bass2jax.partition_id_tensor())
        outs = _bass_exec_p.bind(
            *operands,
            out_avals=tuple(out_avals),
            in_names=tuple(all_in_names),
            out_names=tuple(out_names),
            lowering_input_output_aliases=(),
            sim_require_finite=True,
            sim_require_nnan=True,
            nc=nc,
        )
        return tuple(outs)

    devices = jax.devices()[:NCORES]
    mesh = Mesh(np.asarray(devices), ("core",))
    # x ("xbf") is identical on every core: pass it replicated so only one
    # copy crosses the host->device link; per-core tensors are concat-sharded.
    in_specs = tuple(
        PartitionSpec() if name == "xbf" else PartitionSpec("core")
        for name in in_names
    ) + (PartitionSpec("core"),) * n_outs
    sharded = jax.jit(
        shard_map(
            _body, mesh=mesh,
            in_specs=in_specs,
            out_specs=(PartitionSpec("core"),) * n_outs,
            check_rep=False,
        ),
        keep_unused=True,
    )
    shard_core = NamedSharding(mesh, PartitionSpec("core"))
    shard_repl = NamedSharding(mesh, PartitionSpec())

    def run(in_maps):
        import jax as _jax
        dev_in = []
        for name in in_names:
            if name == "xbf":
                dev_in.append(
                    _jax.device_put(np.asarray(in_maps[0][name]), shard_repl))
            else:
                a = np.concatenate(
                    [np.asarray(in_maps[c][name]) for c in range(NCORES)], axis=0)
                dev_in.append(_jax.device_put(a, shard_core))
        dev_zero = [
            _jax.device_put(
                np.zeros((NCORES * z.shape[0], *z.shape[1:]), z.dtype), shard_core)
            for z in zero_outs
        ]
        out = sharded(*dev_in, *dev_zero)
        return [
            {name: np.asarray(out[i]).reshape(NCORES, *zero_outs[i].shape)[c]
             for i, name in enumerate(out_names)}
            for c in range(NCORES)
        ]

    return run


def kernel(x, scales, bias, weight_int8, col_indices, group_size):
    global _RUNNER
    in_maps = make_in_maps(x, scales, bias, weight_int8, col_indices, group_size)
    if _RUNNER is None:
        _RUNNER = _make_runner()
    results = _RUNNER(in_maps)
    y = np.concatenate(
        [results[c]["y"].astype(np.float32) for c in range(NCORES)], axis=1)
    y += np.asarray(bias, dtype=np.float32)
    return np.ascontiguousarray(y.reshape(B, S, N))
